# revision 21
# baseline (speedup 1.0000x reference)
"""Trainium2 Bass kernel for a 16-head causal MHA block (B=4, S=2048, D=1024).

Sharding: 8 cores = 4 batches x 2 head-groups (8 heads each).
Per-core dataflow (all feature-major / "transposed" layouts):
  qpT[d,s] = WqT.T @ qT   (+bq)        kpT[d,s] = WkT.T @ kT
  kp [s,d] = kT.T @ WkT   (kh output)  vp [s,d] = vT.T @ WvT  (vh output + PV)
  scoresT[k,q] = kpT.T(d) @ qpT  (per head, K=64, causal-narrowed)
  pT = exp(scoresT/8 + tri)      (no max-subtract: scores ~ N(0,1))
  pv[d,q] & rowsum via ones-augmented stationary [v|1] (M=128)
  x[d,q] = pv * (1/rowsum)       out_part[s,:] = x.T(d) @ WoT
Biases bk, bv, bo are restored on the host (softmax-invariant / linear).
Emission is interleaved si-quarter -> attention qi -> outproj qi so the
Tile scheduler can overlap projections (PE) with attention (ACT-paced).
"""
import numpy as np

import concourse.bass as bass
import concourse.mybir as mybir
import concourse.tile as tile
from concourse import bacc
from concourse.bass_utils import run_bass_kernel_spmd

F32 = mybir.dt.float32
F32R = mybir.dt.float32r
AF = mybir.ActivationFunctionType

B, S, D, H = 4, 2048, 1024, 16
HD = 64
NCORES = 8
HG = 8      # heads per core
DG = 512    # projected dims per core (head-group)
E = 1024    # input feature dim
SCALE = 0.125       # 1/sqrt(HD)
NEGMASK = -8.0e5    # pre-scale additive mask: exp((s + NEGMASK)/8) == 0.0 in f32

QW = 512    # q-chunk width for attention
NQC = S // QW   # 4
NKC = S // 128  # 16


def _mm(nc, out, lhsT, rhs, start, stop):
    nc.tensor.matmul(out, lhsT=lhsT, rhs=rhs, start=start, stop=stop)


def _emit(nc):
    qT = nc.dram_tensor("qT", [E, S], F32R, kind="ExternalInput").ap()
    kT = nc.dram_tensor("kT", [E, S], F32R, kind="ExternalInput").ap()
    vT = nc.dram_tensor("vT", [E, S], F32R, kind="ExternalInput").ap()
    WqT = nc.dram_tensor("WqT", [E, DG], F32R, kind="ExternalInput").ap()
    WkT = nc.dram_tensor("WkT", [E, DG], F32R, kind="ExternalInput").ap()
    WvT = nc.dram_tensor("WvT", [E, DG], F32R, kind="ExternalInput").ap()
    WoT = nc.dram_tensor("WoT", [DG, D], F32R, kind="ExternalInput").ap()
    bqv = nc.dram_tensor("bq", [DG], F32, kind="ExternalInput").ap()
    tri = nc.dram_tensor("tri", [128, 128], F32, kind="ExternalInput").ap()
    outp = nc.dram_tensor("outp", [S, D], F32, kind="ExternalOutput").ap()
    kh_o = nc.dram_tensor("kh_o", [HG, S, HD], F32, kind="ExternalOutput").ap()
    vh_o = nc.dram_tensor("vh_o", [HG, S, HD], F32, kind="ExternalOutput").ap()

    with tile.TileContext(nc) as tc:
        with (
            tc.tile_pool(name="persist", bufs=1) as pp,
            tc.tile_pool(name="win", bufs=1) as wp,
            tc.tile_pool(name="ain", bufs=2) as ap_,
            tc.tile_pool(name="stage", bufs=2) as stg,
            tc.tile_pool(name="pt", bufs=3) as ptp,
            tc.tile_pool(name="rc", bufs=1) as rcp,
            tc.tile_pool(name="psA", bufs=2, space="PSUM") as psA,
            tc.tile_pool(name="psS", bufs=3, space="PSUM") as psS,
            tc.tile_pool(name="psV", bufs=3, space="PSUM") as psV,
        ):
            # ---- persistent tiles ----
            qpT = [pp.tile([128, S], F32R, tag=f"qpT{p}", name=f"qpT{p}") for p in range(4)]
            kpT = [pp.tile([128, S], F32R, tag=f"kpT{p}", name=f"kpT{p}") for p in range(4)]
            # vpa: per s-chunk of 128, interleaved per pair: [v_even|ones|v_odd]
            vpa = [pp.tile([128, 768], F32R, tag=f"vpa{i}", name=f"vpa{i}") for i in range(NKC)]
            x = [pp.tile([128, S], F32R, tag=f"x{p}", name=f"x{p}") for p in range(4)]
            tri_sb = pp.tile([128, 128], F32, tag="tri")
            bq_sb = pp.tile([128, 4], F32, tag="bq")
            ones_sb = pp.tile([128, 256], F32, tag="ones")

            nc.sync.dma_start(out=tri_sb[:], in_=tri)
            nc.sync.dma_start(out=bq_sb[:], in_=bqv.rearrange("(c p) -> p c", p=128))
            nc.gpsimd.memset(ones_sb[:], 1.0)
            for i in range(NKC):
                nc.vector.tensor_copy(
                    vpa[i][:].rearrange("p (j b) -> p j b", j=4)[:, :, 64:128],
                    ones_sb[:].rearrange("p (j b) -> p j b", j=4),
                )

            def load_w(src, pref):
                tiles = []
                for e in range(8):
                    t = wp.tile([128, DG], F32R, tag=f"w{e}", name=f"w{pref}{e}")
                    nc.sync.dma_start(out=t[:], in_=src[e * 128 : (e + 1) * 128, :])
                    tiles.append(t)
                return tiles


            def load_quarter(src, si):
                tiles = []
                for e in range(8):
                    t = ap_.tile([128, QW], F32R, tag=f"a{e}", name=f"a{e}")
                    nc.sync.dma_start(
                        out=t[:],
                        in_=src[e * 128 : (e + 1) * 128, si * QW : (si + 1) * QW],
                    )
                    tiles.append(t)
                return tiles

            # ---------------- phase emitters ----------------
            def projT_block(inp, w, out_tiles, si, bias=None):
                for dc in range(4):
                    ps = psA.tile([128, QW], F32, tag="ps")
                    for e in range(8):
                        _mm(nc, ps[:], w[e][:, dc * 128 : (dc + 1) * 128],
                            inp[e][:], start=(e == 0), stop=(e == 7))
                    dst = out_tiles[dc][:, si * QW : (si + 1) * QW]
                    if bias is not None:
                        nc.vector.tensor_scalar_add(dst, ps[:], bias[:, dc : dc + 1])
                    else:
                        nc.vector.tensor_copy(dst, ps[:])

            def nat_block(inp, w, si, kind):
                for sub in range(4):
                    sc = si * 4 + sub
                    ps = psA.tile([128, DG], F32, tag="ps")
                    for e in range(8):
                        _mm(nc, ps[:], inp[e][:, sub * 128 : (sub + 1) * 128],
                            w[e][:], start=(e == 0), stop=(e == 7))
                    if kind == "k":
                        st = stg.tile([128, DG], F32, tag="st")
                        nc.vector.tensor_copy(st[:], ps[:])
                        nc.gpsimd.dma_start(
                            out=kh_o.rearrange("h s c -> s h c")[sc * 128 : (sc + 1) * 128],
                            in_=st[:].rearrange("p (h c) -> p h c", c=HD),
                        )
                    else:
                        nc.vector.tensor_copy(
                            vpa[sc][:].rearrange("p (j b) -> p j b", j=4)[:, :, 0:64],
                            ps[:].rearrange("p (j b) -> p j b", j=4)[:, :, 0:64],
                        )
                        nc.vector.tensor_copy(
                            vpa[sc][:].rearrange("p (j b) -> p j b", j=4)[:, :, 128:192],
                            ps[:].rearrange("p (j b) -> p j b", j=4)[:, :, 64:128],
                        )
                        stv = stg.tile([128, DG], F32, tag="st")
                        nc.vector.tensor_copy(stv[:], ps[:])
                        nc.gpsimd.dma_start(
                            out=vh_o.rearrange("h s c -> s h c")[sc * 128 : (sc + 1) * 128],
                            in_=stv[:].rearrange("p (h c) -> p h c", c=HD),
                        )

            def attn_unit(p, qi):
                q0 = qi * QW
                nkc = (q0 + QW) // 128
                pv0 = psV.tile([128, QW], F32, tag="pv")
                pv1 = psV.tile([128, QW], F32, tag="pv")
                pend = []  # software pipeline: PV trails scores by one kc
                for kc in range(nkc):
                    k0 = kc * 128
                    qs = max(0, k0 - q0)
                    w = QW - qs
                    s0 = psS.tile([128, QW], F32, tag="sc")
                    s1 = psS.tile([128, QW], F32, tag="sc")
                    _mm(nc, s0[:, qs:QW], kpT[p][0:64, k0 : k0 + 128],
                        qpT[p][0:64, q0 + qs : q0 + QW], start=True, stop=True)
                    _mm(nc, s1[:, qs:QW], kpT[p][64:128, k0 : k0 + 128],
                        qpT[p][64:128, q0 + qs : q0 + QW], start=True, stop=True)
                    if k0 >= q0:
                        nc.vector.tensor_add(s0[:, qs : qs + 128], s0[:, qs : qs + 128], tri_sb[:])
                        nc.vector.tensor_add(s1[:, qs : qs + 128], s1[:, qs : qs + 128], tri_sb[:])
                    pt0 = ptp.tile([128, QW], F32R, tag="pt")
                    pt1 = ptp.tile([128, QW], F32R, tag="pt")
                    nc.scalar.activation(pt0[:, 0:w], s0[:, qs:QW], AF.Exp, scale=SCALE)
                    nc.scalar.activation(pt1[:, 0:w], s1[:, qs:QW], AF.Exp, scale=SCALE)
                    pend.append((kc, qs, w, pt0, pt1))
                    if len(pend) > 1:
                        _pv_step(p, pv0, pv1, pend.pop(0), nkc)
                while pend:
                    _pv_step(p, pv0, pv1, pend.pop(0), nkc)
                # normalization / eviction
                for j, pv in ((0, pv0), (1, pv1)):
                    lo, hi = (0, 64) if j == 0 else (64, 128)
                    slo, shi = (64, 128) if j == 0 else (0, 64)
                    r = rcp.tile([128, QW], F32, tag="rc")
                    nc.vector.reciprocal_approx_fast(r[:], pv[:])
                    rb = rcp.tile([128, QW], F32, tag="rcb")
                    nc.gpsimd.dma_start(out=rb[lo:hi, :], in_=r[slo:shi, :])
                    nc.vector.tensor_mul(
                        x[p][lo:hi, q0 : q0 + QW], pv[lo:hi, :], rb[lo:hi, :]
                    )

            def _pv_step(p, pv0, pv1, item, nkc):
                kc, qs, w, pt0, pt1 = item
                _mm(nc, pv0[:, qs:QW], vpa[kc][:, 192 * p : 192 * p + 128],
                    pt0[:, 0:w], start=(kc == 0), stop=(kc == nkc - 1))
                _mm(nc, pv1[:, qs:QW], vpa[kc][:, 192 * p + 64 : 192 * p + 192],
                    pt1[:, 0:w], start=(kc == 0), stop=(kc == nkc - 1))

            wo = []

            def load_wo():
                for p4 in range(4):
                    for hf in range(2):
                        t = wp.tile([128, DG], F32R, tag=f"w{p4*2+hf}", name=f"wo{p4*2+hf}")
                        nc.sync.dma_start(
                            out=t[:],
                            in_=WoT[p4 * 128 : (p4 + 1) * 128, hf * 512 : (hf + 1) * 512],
                        )
                        wo.append(t)

            def outproj_block(qi):
                for sub in range(4):
                    sc = qi * 4 + sub
                    for hf in range(2):
                        ps = psA.tile([128, DG], F32, tag="ps")
                        for p4 in range(4):
                            _mm(nc, ps[:], x[p4][:, sc * 128 : (sc + 1) * 128],
                                wo[p4 * 2 + hf][:], start=(p4 == 0), stop=(p4 == 3))
                        st = stg.tile([128, DG], F32, tag="st")
                        nc.vector.tensor_copy(st[:], ps[:])
                        nc.gpsimd.dma_start(
                            out=outp[sc * 128 : (sc + 1) * 128, hf * 512 : (hf + 1) * 512],
                            in_=st[:],
                        )

            # ---------------- phase-major emission ----------------
            wq = load_w(WqT, "q")
            for si in range(4):
                qin = load_quarter(qT, si)
                projT_block(qin, wq, qpT, si, bias=bq_sb)
            wk = load_w(WkT, "k")
            for si in range(4):
                kin = load_quarter(kT, si)
                projT_block(kin, wk, kpT, si)
                nat_block(kin, wk, si, "k")
            wv = load_w(WvT, "v")
            for si in range(4):
                vin = load_quarter(vT, si)
                nat_block(vin, wv, si, "v")
            load_wo()
            for p in range(4):
                for qi in range(NQC):
                    attn_unit(p, qi)
            for qi in range(NQC):
                outproj_block(qi)
    return nc


_NC_CACHE = {}
_LAST_IN_MAPS = None


def _get_nc():
    if "nc" not in _NC_CACHE:
        nc = bacc.Bacc(
            "TRN2", target_bir_lowering=False, debug=False, num_devices=NCORES
        )
        _emit(nc)
        nc.compile()
        _NC_CACHE["nc"] = nc
    return _NC_CACHE["nc"]


def kernel(q, k, v, mask, Wq, bq, Wk, bk, Wv, bv, Wo, bo):
    q = np.asarray(q, np.float32)
    k = np.asarray(k, np.float32)
    v = np.asarray(v, np.float32)
    Wq = np.asarray(Wq, np.float32)
    Wk = np.asarray(Wk, np.float32)
    Wv = np.asarray(Wv, np.float32)
    Wo = np.asarray(Wo, np.float32)
    bq = np.asarray(bq, np.float32)
    bk = np.asarray(bk, np.float32)
    bv = np.asarray(bv, np.float32)
    bo = np.asarray(bo, np.float32)

    nc = _get_nc()

    tri = np.zeros((128, 128), np.float32)
    iu = np.triu_indices(128, k=1)
    # pT layout is [k, q]: masked iff q < k -> strictly lower triangle of [k, q]
    tri[(iu[1], iu[0])] = NEGMASK

    WqT_f = np.ascontiguousarray(Wq.T)  # [E, D]
    WkT_f = np.ascontiguousarray(Wk.T)
    WvT_f = np.ascontiguousarray(Wv.T)
    WoT_f = np.ascontiguousarray(Wo.T)  # [D(mid), D(out)]

    in_maps = []
    for core in range(NCORES):
        b, g = core // 2, core % 2
        sl = slice(g * DG, (g + 1) * DG)
        in_maps.append(
            {
                "qT": np.ascontiguousarray(q[b].T),
                "kT": np.ascontiguousarray(k[b].T),
                "vT": np.ascontiguousarray(v[b].T),
                "WqT": np.ascontiguousarray(WqT_f[:, sl]),
                "WkT": np.ascontiguousarray(WkT_f[:, sl]),
                "WvT": np.ascontiguousarray(WvT_f[:, sl]),
                "WoT": np.ascontiguousarray(WoT_f[sl, :]),
                "bq": np.ascontiguousarray(bq[sl]),
                "tri": tri,
            }
        )

    global _LAST_IN_MAPS
    _LAST_IN_MAPS = in_maps
    res = run_bass_kernel_spmd(nc, in_maps, list(range(NCORES)))

    out = np.zeros((B, S, D), np.float32)
    kh = np.zeros((B, H, S, HD), np.float32)
    vh = np.zeros((B, H, S, HD), np.float32)
    for core in range(NCORES):
        b, g = core // 2, core % 2
        r = res.results[core]
        out[b] += r["outp"]
        kh[b, g * HG : (g + 1) * HG] = r["kh_o"]
        vh[b, g * HG : (g + 1) * HG] = r["vh_o"]
    # restore biases dropped on-device (linear / softmax-invariant)
    out += bo[None, None, :]
    out += (bv @ Wo.T)[None, None, :]
    kh += bk.reshape(H, 1, HD)
    vh += bv.reshape(H, 1, HD)
    return out, kh, vh


# revision 22
# speedup vs baseline: 1.0579x; 1.0579x over previous
"""Trainium2 Bass kernel for a 16-head causal MHA block (B=4, S=2048, D=1024).

Sharding: 8 cores = 4 batches x 2 head-groups (8 heads each).
Per-core dataflow (all feature-major / "transposed" layouts):
  qpT[d,s] = WqT.T @ qT   (+bq)        kpT[d,s] = WkT.T @ kT
  kp [s,d] = kT.T @ WkT   (kh output)  vp [s,d] = vT.T @ WvT  (vh output + PV)
  scoresT[k,q] = kpT.T(d) @ qpT  (per head, K=64, causal-narrowed)
  pT = exp(scoresT/8 + tri)      (no max-subtract: scores ~ N(0,1))
  pv[d,q] & rowsum via ones-augmented stationary [v|1] (M=128)
  x[d,q] = pv * (1/rowsum)       out_part[s,:] = x.T(d) @ WoT
Biases bk, bv, bo are restored on the host (softmax-invariant / linear).
Emission is interleaved si-quarter -> attention qi -> outproj qi so the
Tile scheduler can overlap projections (PE) with attention (ACT-paced).
"""
import numpy as np

import concourse.bass as bass
import concourse.mybir as mybir
import concourse.tile as tile
from concourse import bacc
from concourse.bass_utils import run_bass_kernel_spmd

F32 = mybir.dt.float32
F32R = mybir.dt.float32r
AF = mybir.ActivationFunctionType

B, S, D, H = 4, 2048, 1024, 16
HD = 64
NCORES = 8
HG = 8      # heads per core
DG = 512    # projected dims per core (head-group)
E = 1024    # input feature dim
SCALE = 0.125       # 1/sqrt(HD)
NEGMASK = -8.0e5    # pre-scale additive mask: exp((s + NEGMASK)/8) == 0.0 in f32

QW = 512    # q-chunk width for attention
NQC = S // QW   # 4
NKC = S // 128  # 16


def _mm(nc, out, lhsT, rhs, start, stop):
    nc.tensor.matmul(out, lhsT=lhsT, rhs=rhs, start=start, stop=stop)


def _emit(nc):
    qT = nc.dram_tensor("qT", [E, S], F32R, kind="ExternalInput").ap()
    kT = nc.dram_tensor("kT", [E, S], F32R, kind="ExternalInput").ap()
    vT = nc.dram_tensor("vT", [E, S], F32R, kind="ExternalInput").ap()
    WqT = nc.dram_tensor("WqT", [E, DG], F32R, kind="ExternalInput").ap()
    WkT = nc.dram_tensor("WkT", [E, DG], F32R, kind="ExternalInput").ap()
    WvT = nc.dram_tensor("WvT", [E, DG], F32R, kind="ExternalInput").ap()
    WoT = nc.dram_tensor("WoT", [DG, D], F32R, kind="ExternalInput").ap()
    bqv = nc.dram_tensor("bq", [DG], F32, kind="ExternalInput").ap()
    tri = nc.dram_tensor("tri", [128, 128], F32, kind="ExternalInput").ap()
    outp = nc.dram_tensor("outp", [S, D], F32, kind="ExternalOutput").ap()
    kh_o = nc.dram_tensor("kh_o", [HG, S, HD], F32, kind="ExternalOutput").ap()
    vh_o = nc.dram_tensor("vh_o", [HG, S, HD], F32, kind="ExternalOutput").ap()

    with tile.TileContext(nc) as tc:
        with (
            tc.tile_pool(name="persist", bufs=1) as pp,
            tc.tile_pool(name="win", bufs=1) as wp,
            tc.tile_pool(name="stage", bufs=2) as stg,
            tc.tile_pool(name="psS", bufs=5, space="PSUM") as psS,
            tc.tile_pool(name="psV", bufs=3, space="PSUM") as psV,
        ):
            # ---- persistent tiles ----
            qpT = [pp.tile([128, S], F32R, tag=f"qpT{p}", name=f"qpT{p}") for p in range(4)]
            kpT = [pp.tile([128, S], F32R, tag=f"kpT{p}", name=f"kpT{p}") for p in range(4)]
            # vpa: per s-chunk of 128, interleaved per pair: [v_even|ones|v_odd]
            vpa = [pp.tile([128, 768], F32R, tag=f"vpa{i}", name=f"vpa{i}") for i in range(NKC)]
            x = [pp.tile([128, S], F32R, tag=f"x{p}", name=f"x{p}") for p in range(4)]
            tri_sb = pp.tile([128, 128], F32, tag="tri")
            bq_sb = pp.tile([128, 4], F32, tag="bq")
            ones_sb = pp.tile([128, 256], F32, tag="ones")

            nc.sync.dma_start(out=tri_sb[:], in_=tri)
            nc.sync.dma_start(out=bq_sb[:], in_=bqv.rearrange("(c p) -> p c", p=128))
            nc.gpsimd.memset(ones_sb[:], 1.0)
            for i in range(NKC):
                nc.vector.tensor_copy(
                    vpa[i][:].rearrange("p (j b) -> p j b", j=4)[:, :, 64:128],
                    ones_sb[:].rearrange("p (j b) -> p j b", j=4),
                )

            def load_w(src, pref):
                tiles = []
                for e in range(8):
                    t = wp.tile([128, DG], F32R, tag=f"w{e}", name=f"w{pref}{e}")
                    nc.sync.dma_start(out=t[:], in_=src[e * 128 : (e + 1) * 128, :])
                    tiles.append(t)
                return tiles


            nonlocal_holder = {}

            def load_quarter(src, si):
                tiles = []
                for e in range(8):
                    t = nonlocal_holder["ap_"].tile([128, QW], F32R, tag=f"a{e}", name=f"a{e}")
                    nc.sync.dma_start(
                        out=t[:],
                        in_=src[e * 128 : (e + 1) * 128, si * QW : (si + 1) * QW],
                    )
                    tiles.append(t)
                return tiles

            # ---------------- phase emitters ----------------
            def projT_block(inp, w, out_tiles, si, bias=None):
                for dc in range(4):
                    ps = psS.tile([128, QW], F32, tag="sc")
                    for e in range(8):
                        _mm(nc, ps[:], w[e][:, dc * 128 : (dc + 1) * 128],
                            inp[e][:], start=(e == 0), stop=(e == 7))
                    dst = out_tiles[dc][:, si * QW : (si + 1) * QW]
                    if bias is not None:
                        nc.vector.tensor_scalar_add(dst, ps[:], bias[:, dc : dc + 1])
                    else:
                        nc.vector.tensor_copy(dst, ps[:])

            def nat_block(inp, w, si, kind):
                for sub in range(4):
                    sc = si * 4 + sub
                    ps = psS.tile([128, DG], F32, tag="sc")
                    for e in range(8):
                        _mm(nc, ps[:], inp[e][:, sub * 128 : (sub + 1) * 128],
                            w[e][:], start=(e == 0), stop=(e == 7))
                    if kind == "k":
                        st = stg.tile([128, DG], F32, tag="st")
                        nc.vector.tensor_copy(st[:], ps[:])
                        nc.gpsimd.dma_start(
                            out=kh_o.rearrange("h s c -> s h c")[sc * 128 : (sc + 1) * 128],
                            in_=st[:].rearrange("p (h c) -> p h c", c=HD),
                        )
                    else:
                        nc.vector.tensor_copy(
                            vpa[sc][:].rearrange("p (j b) -> p j b", j=4)[:, :, 0:64],
                            ps[:].rearrange("p (j b) -> p j b", j=4)[:, :, 0:64],
                        )
                        nc.vector.tensor_copy(
                            vpa[sc][:].rearrange("p (j b) -> p j b", j=4)[:, :, 128:192],
                            ps[:].rearrange("p (j b) -> p j b", j=4)[:, :, 64:128],
                        )
                        stv = stg.tile([128, DG], F32, tag="st")
                        nc.vector.tensor_copy(stv[:], ps[:])
                        nc.gpsimd.dma_start(
                            out=vh_o.rearrange("h s c -> s h c")[sc * 128 : (sc + 1) * 128],
                            in_=stv[:].rearrange("p (h c) -> p h c", c=HD),
                        )

            def attn_unit(p, qi, ptp, rcp):
                q0 = qi * QW
                nkc = (q0 + QW) // 128
                pv0 = psV.tile([128, QW], F32, tag="pv")
                pv1 = psV.tile([128, QW], F32, tag="pv")
                pend = []  # software pipeline: PV trails scores by one kc
                for kc in range(nkc):
                    k0 = kc * 128
                    qs = max(0, k0 - q0)
                    w = QW - qs
                    s0 = psS.tile([128, QW], F32, tag="sc")
                    s1 = psS.tile([128, QW], F32, tag="sc")
                    _mm(nc, s0[:, qs:QW], kpT[p][0:64, k0 : k0 + 128],
                        qpT[p][0:64, q0 + qs : q0 + QW], start=True, stop=True)
                    _mm(nc, s1[:, qs:QW], kpT[p][64:128, k0 : k0 + 128],
                        qpT[p][64:128, q0 + qs : q0 + QW], start=True, stop=True)
                    if k0 >= q0:
                        nc.vector.tensor_add(s0[:, qs : qs + 128], s0[:, qs : qs + 128], tri_sb[:])
                        nc.vector.tensor_add(s1[:, qs : qs + 128], s1[:, qs : qs + 128], tri_sb[:])
                    pt0 = ptp.tile([128, QW], F32R, tag="pt", name="pt0")
                    pt1 = ptp.tile([128, QW], F32R, tag="pt", name="pt1")
                    nc.scalar.activation(pt0[:, 0:w], s0[:, qs:QW], AF.Exp, scale=SCALE)
                    nc.scalar.activation(pt1[:, 0:w], s1[:, qs:QW], AF.Exp, scale=SCALE)
                    pend.append((kc, qs, w, pt0, pt1))
                    if len(pend) > 1:
                        _pv_step(p, pv0, pv1, pend.pop(0), nkc)
                while pend:
                    _pv_step(p, pv0, pv1, pend.pop(0), nkc)
                # normalization / eviction
                for j, pv in ((0, pv0), (1, pv1)):
                    lo, hi = (0, 64) if j == 0 else (64, 128)
                    slo, shi = (64, 128) if j == 0 else (0, 64)
                    r = rcp.tile([128, QW], F32, tag="rc")
                    nc.vector.reciprocal_approx_fast(r[:], pv[:])
                    rb = rcp.tile([128, QW], F32, tag="rcb")
                    nc.gpsimd.dma_start(out=rb[lo:hi, :], in_=r[slo:shi, :])
                    nc.vector.tensor_mul(
                        x[p][lo:hi, q0 : q0 + QW], pv[lo:hi, :], rb[lo:hi, :]
                    )

            def _pv_step(p, pv0, pv1, item, nkc):
                kc, qs, w, pt0, pt1 = item
                _mm(nc, pv0[:, qs:QW], vpa[kc][:, 192 * p : 192 * p + 128],
                    pt0[:, 0:w], start=(kc == 0), stop=(kc == nkc - 1))
                _mm(nc, pv1[:, qs:QW], vpa[kc][:, 192 * p + 64 : 192 * p + 192],
                    pt1[:, 0:w], start=(kc == 0), stop=(kc == nkc - 1))

            wo = []

            def load_wo():
                for p4 in range(4):
                    for hf in range(2):
                        t = wp.tile([128, DG], F32R, tag=f"w{p4*2+hf}", name=f"wo{p4*2+hf}")
                        nc.sync.dma_start(
                            out=t[:],
                            in_=WoT[p4 * 128 : (p4 + 1) * 128, hf * 512 : (hf + 1) * 512],
                        )
                        wo.append(t)

            def outproj_block(qi):
                for sub in range(4):
                    sc = qi * 4 + sub
                    for hf in range(2):
                        ps = psS.tile([128, DG], F32, tag="sc")
                        for p4 in range(4):
                            _mm(nc, ps[:], x[p4][:, sc * 128 : (sc + 1) * 128],
                                wo[p4 * 2 + hf][:], start=(p4 == 0), stop=(p4 == 3))
                        st = stg.tile([128, DG], F32, tag="st")
                        nc.vector.tensor_copy(st[:], ps[:])
                        nc.gpsimd.dma_start(
                            out=outp[sc * 128 : (sc + 1) * 128, hf * 512 : (hf + 1) * 512],
                            in_=st[:],
                        )

            # ---------------- phase-major emission ----------------
            with tc.tile_pool(name="ain", bufs=2) as ap2:
                nonlocal_holder["ap_"] = ap2
                wq = load_w(WqT, "q")
                for si in range(4):
                    qin = load_quarter(qT, si)
                    projT_block(qin, wq, qpT, si, bias=bq_sb)
                wk = load_w(WkT, "k")
                for si in range(4):
                    kin = load_quarter(kT, si)
                    projT_block(kin, wk, kpT, si)
                    nat_block(kin, wk, si, "k")
                wv = load_w(WvT, "v")
                for si in range(4):
                    vin = load_quarter(vT, si)
                    nat_block(vin, wv, si, "v")
            load_wo()
            with (
                tc.tile_pool(name="pt", bufs=6) as ptp,
                tc.tile_pool(name="rc", bufs=2) as rcp,
            ):
                for p in range(4):
                    for qi in range(NQC):
                        attn_unit(p, qi, ptp, rcp)
                for qi in range(NQC):
                    outproj_block(qi)
    return nc


_NC_CACHE = {}
_LAST_IN_MAPS = None


def _get_nc():
    if "nc" not in _NC_CACHE:
        nc = bacc.Bacc(
            "TRN2", target_bir_lowering=False, debug=False, num_devices=NCORES
        )
        _emit(nc)
        nc.compile()
        _NC_CACHE["nc"] = nc
    return _NC_CACHE["nc"]


def kernel(q, k, v, mask, Wq, bq, Wk, bk, Wv, bv, Wo, bo):
    q = np.asarray(q, np.float32)
    k = np.asarray(k, np.float32)
    v = np.asarray(v, np.float32)
    Wq = np.asarray(Wq, np.float32)
    Wk = np.asarray(Wk, np.float32)
    Wv = np.asarray(Wv, np.float32)
    Wo = np.asarray(Wo, np.float32)
    bq = np.asarray(bq, np.float32)
    bk = np.asarray(bk, np.float32)
    bv = np.asarray(bv, np.float32)
    bo = np.asarray(bo, np.float32)

    nc = _get_nc()

    tri = np.zeros((128, 128), np.float32)
    iu = np.triu_indices(128, k=1)
    # pT layout is [k, q]: masked iff q < k -> strictly lower triangle of [k, q]
    tri[(iu[1], iu[0])] = NEGMASK

    WqT_f = np.ascontiguousarray(Wq.T)  # [E, D]
    WkT_f = np.ascontiguousarray(Wk.T)
    WvT_f = np.ascontiguousarray(Wv.T)
    WoT_f = np.ascontiguousarray(Wo.T)  # [D(mid), D(out)]

    in_maps = []
    for core in range(NCORES):
        b, g = core // 2, core % 2
        sl = slice(g * DG, (g + 1) * DG)
        in_maps.append(
            {
                "qT": np.ascontiguousarray(q[b].T),
                "kT": np.ascontiguousarray(k[b].T),
                "vT": np.ascontiguousarray(v[b].T),
                "WqT": np.ascontiguousarray(WqT_f[:, sl]),
                "WkT": np.ascontiguousarray(WkT_f[:, sl]),
                "WvT": np.ascontiguousarray(WvT_f[:, sl]),
                "WoT": np.ascontiguousarray(WoT_f[sl, :]),
                "bq": np.ascontiguousarray(bq[sl]),
                "tri": tri,
            }
        )

    global _LAST_IN_MAPS
    _LAST_IN_MAPS = in_maps
    res = run_bass_kernel_spmd(nc, in_maps, list(range(NCORES)))

    out = np.zeros((B, S, D), np.float32)
    kh = np.zeros((B, H, S, HD), np.float32)
    vh = np.zeros((B, H, S, HD), np.float32)
    for core in range(NCORES):
        b, g = core // 2, core % 2
        r = res.results[core]
        out[b] += r["outp"]
        kh[b, g * HG : (g + 1) * HG] = r["kh_o"]
        vh[b, g * HG : (g + 1) * HG] = r["vh_o"]
    # restore biases dropped on-device (linear / softmax-invariant)
    out += bo[None, None, :]
    out += (bv @ Wo.T)[None, None, :]
    kh += bk.reshape(H, 1, HD)
    vh += bv.reshape(H, 1, HD)
    return out, kh, vh


# revision 23
# speedup vs baseline: 1.0606x; 1.0026x over previous
"""Trainium2 Bass kernel for a 16-head causal MHA block (B=4, S=2048, D=1024).

Sharding: 8 cores = 4 batches x 2 head-groups (8 heads each).
Per-core dataflow (all feature-major / "transposed" layouts):
  qpT[d,s] = WqT.T @ qT   (+bq)        kpT[d,s] = WkT.T @ kT
  kp [s,d] = kT.T @ WkT   (kh output)  vp [s,d] = vT.T @ WvT  (vh output + PV)
  scoresT[k,q] = kpT.T(d) @ qpT  (per head, K=64, causal-narrowed)
  pT = exp(scoresT/8 + tri)      (no max-subtract: scores ~ N(0,1))
  pv[d,q] & rowsum via ones-augmented stationary [v|1] (M=128)
  x[d,q] = pv * (1/rowsum)       out_part[s,:] = x.T(d) @ WoT
Biases bk, bv, bo are restored on the host (softmax-invariant / linear).
Emission is interleaved si-quarter -> attention qi -> outproj qi so the
Tile scheduler can overlap projections (PE) with attention (ACT-paced).
"""
import numpy as np

import concourse.bass as bass
import concourse.mybir as mybir
import concourse.tile as tile
from concourse import bacc
from concourse.bass_utils import run_bass_kernel_spmd

F32 = mybir.dt.float32
F32R = mybir.dt.float32r
AF = mybir.ActivationFunctionType

B, S, D, H = 4, 2048, 1024, 16
HD = 64
NCORES = 8
HG = 8      # heads per core
DG = 512    # projected dims per core (head-group)
E = 1024    # input feature dim
SCALE = 0.125       # 1/sqrt(HD)
NEGMASK = -8.0e5    # pre-scale additive mask: exp((s + NEGMASK)/8) == 0.0 in f32

QW = 512    # q-chunk width for attention
NQC = S // QW   # 4
NKC = S // 128  # 16


def _mm(nc, out, lhsT, rhs, start, stop):
    nc.tensor.matmul(out, lhsT=lhsT, rhs=rhs, start=start, stop=stop)


def _emit(nc):
    qT = nc.dram_tensor("qT", [E, S], F32R, kind="ExternalInput").ap()
    kT = nc.dram_tensor("kT", [E, S], F32R, kind="ExternalInput").ap()
    vT = nc.dram_tensor("vT", [E, S], F32R, kind="ExternalInput").ap()
    WqT = nc.dram_tensor("WqT", [E, DG], F32R, kind="ExternalInput").ap()
    WkT = nc.dram_tensor("WkT", [E, DG], F32R, kind="ExternalInput").ap()
    WvT = nc.dram_tensor("WvT", [E, DG], F32R, kind="ExternalInput").ap()
    WoT = nc.dram_tensor("WoT", [DG, D], F32R, kind="ExternalInput").ap()
    bqv = nc.dram_tensor("bq", [DG], F32, kind="ExternalInput").ap()
    tri = nc.dram_tensor("tri", [128, 128], F32, kind="ExternalInput").ap()
    outp = nc.dram_tensor("outp", [S, D], F32, kind="ExternalOutput").ap()
    kh_o = nc.dram_tensor("kh_o", [HG, S, HD], F32, kind="ExternalOutput").ap()
    vh_o = nc.dram_tensor("vh_o", [HG, S, HD], F32, kind="ExternalOutput").ap()

    with tile.TileContext(nc) as tc:
        with (
            tc.tile_pool(name="persist", bufs=1) as pp,
            tc.tile_pool(name="win", bufs=1) as wp,
            tc.tile_pool(name="stage", bufs=2) as stg,
            tc.tile_pool(name="psS", bufs=5, space="PSUM") as psS,
            tc.tile_pool(name="psV", bufs=3, space="PSUM") as psV,
        ):
            # ---- persistent tiles ----
            qpT = [pp.tile([128, S], F32R, tag=f"qpT{p}", name=f"qpT{p}") for p in range(4)]
            kpT = [pp.tile([128, S], F32R, tag=f"kpT{p}", name=f"kpT{p}") for p in range(4)]
            # vpa: per s-chunk of 128, interleaved per pair: [v_even|ones|v_odd]
            vpa = [pp.tile([128, 768], F32R, tag=f"vpa{i}", name=f"vpa{i}") for i in range(NKC)]
            x = [pp.tile([128, S], F32R, tag=f"x{p}", name=f"x{p}") for p in range(4)]
            tri_sb = pp.tile([128, 128], F32, tag="tri")
            bq_sb = pp.tile([128, 4], F32, tag="bq")
            ones_sb = pp.tile([128, 256], F32, tag="ones")

            nc.sync.dma_start(out=tri_sb[:], in_=tri)
            nc.sync.dma_start(out=bq_sb[:], in_=bqv.rearrange("(c p) -> p c", p=128))
            nc.gpsimd.memset(ones_sb[:], 1.0)
            for i in range(NKC):
                nc.vector.tensor_copy(
                    vpa[i][:].rearrange("p (j b) -> p j b", j=4)[:, :, 64:128],
                    ones_sb[:].rearrange("p (j b) -> p j b", j=4),
                )

            def load_w(src, pref):
                tiles = []
                for e in range(8):
                    t = wp.tile([128, DG], F32R, tag=f"w{e}", name=f"w{pref}{e}")
                    nc.sync.dma_start(out=t[:], in_=src[e * 128 : (e + 1) * 128, :])
                    tiles.append(t)
                return tiles


            nonlocal_holder = {}

            def load_quarter(src, si):
                tiles = []
                for e in range(8):
                    t = nonlocal_holder["ap_"].tile([128, QW], F32R, tag=f"a{e}", name=f"a{e}")
                    nc.sync.dma_start(
                        out=t[:],
                        in_=src[e * 128 : (e + 1) * 128, si * QW : (si + 1) * QW],
                    )
                    tiles.append(t)
                return tiles

            # ---------------- phase emitters ----------------
            def projT_block(inp, w, out_tiles, si, bias=None):
                for dc in range(4):
                    ps = psS.tile([128, QW], F32, tag="sc")
                    for e in range(8):
                        _mm(nc, ps[:], w[e][:, dc * 128 : (dc + 1) * 128],
                            inp[e][:], start=(e == 0), stop=(e == 7))
                    dst = out_tiles[dc][:, si * QW : (si + 1) * QW]
                    if bias is not None:
                        nc.vector.tensor_scalar_add(dst, ps[:], bias[:, dc : dc + 1])
                    else:
                        nc.vector.tensor_copy(dst, ps[:])

            def nat_block(inp, w, si, kind):
                for sub in range(4):
                    sc = si * 4 + sub
                    ps = psS.tile([128, DG], F32, tag="sc")
                    for e in range(8):
                        _mm(nc, ps[:], inp[e][:, sub * 128 : (sub + 1) * 128],
                            w[e][:], start=(e == 0), stop=(e == 7))
                    if kind == "k":
                        st = stg.tile([128, DG], F32, tag="st")
                        nc.vector.tensor_copy(st[:], ps[:])
                        nc.gpsimd.dma_start(
                            out=kh_o.rearrange("h s c -> s h c")[sc * 128 : (sc + 1) * 128],
                            in_=st[:].rearrange("p (h c) -> p h c", c=HD),
                        )
                    else:
                        nc.vector.tensor_copy(
                            vpa[sc][:].rearrange("p (j b) -> p j b", j=4)[:, :, 0:64],
                            ps[:].rearrange("p (j b) -> p j b", j=4)[:, :, 0:64],
                        )
                        nc.vector.tensor_copy(
                            vpa[sc][:].rearrange("p (j b) -> p j b", j=4)[:, :, 128:192],
                            ps[:].rearrange("p (j b) -> p j b", j=4)[:, :, 64:128],
                        )
                        stv = stg.tile([128, DG], F32, tag="st")
                        nc.vector.tensor_copy(stv[:], ps[:])
                        nc.gpsimd.dma_start(
                            out=vh_o.rearrange("h s c -> s h c")[sc * 128 : (sc + 1) * 128],
                            in_=stv[:].rearrange("p (h c) -> p h c", c=HD),
                        )

            def attn_unit(p, qi, ptp, rcp):
                q0 = qi * QW
                nkc = (q0 + QW) // 128
                pv0 = psV.tile([128, QW], F32, tag="pv")
                pv1 = psV.tile([128, QW], F32, tag="pv")
                pend = []  # software pipeline: PV trails scores by one kc
                for kc in range(nkc):
                    k0 = kc * 128
                    qs = max(0, k0 - q0)
                    w = QW - qs
                    s0 = psS.tile([128, QW], F32, tag="sc")
                    s1 = psS.tile([128, QW], F32, tag="sc")
                    _mm(nc, s0[:, qs:QW], kpT[p][0:64, k0 : k0 + 128],
                        qpT[p][0:64, q0 + qs : q0 + QW], start=True, stop=True)
                    _mm(nc, s1[:, qs:QW], kpT[p][64:128, k0 : k0 + 128],
                        qpT[p][64:128, q0 + qs : q0 + QW], start=True, stop=True)
                    if k0 >= q0:
                        nc.vector.tensor_add(s0[:, qs : qs + 128], s0[:, qs : qs + 128], tri_sb[:])
                        nc.vector.tensor_add(s1[:, qs : qs + 128], s1[:, qs : qs + 128], tri_sb[:])
                    pt0 = ptp.tile([128, QW], F32R, tag="pt", name="pt0")
                    pt1 = ptp.tile([128, QW], F32R, tag="pt", name="pt1")
                    nc.scalar.activation(pt0[:, 0:w], s0[:, qs:QW], AF.Exp, scale=SCALE)
                    nc.scalar.activation(pt1[:, 0:w], s1[:, qs:QW], AF.Exp, scale=SCALE)
                    pend.append((kc, qs, w, pt0, pt1))
                    if len(pend) > 1:
                        _pv_step(p, pv0, pv1, pend.pop(0), nkc)
                while pend:
                    _pv_step(p, pv0, pv1, pend.pop(0), nkc)
                # normalization / eviction
                for j, pv in ((0, pv0), (1, pv1)):
                    lo, hi = (0, 64) if j == 0 else (64, 128)
                    slo, shi = (64, 128) if j == 0 else (0, 64)
                    r = rcp.tile([128, QW], F32, tag="rc")
                    nc.vector.reciprocal_approx_fast(r[:], pv[:])
                    rb = rcp.tile([128, QW], F32, tag="rcb")
                    nc.gpsimd.dma_start(out=rb[lo:hi, :], in_=r[slo:shi, :])
                    nc.vector.tensor_mul(
                        x[p][lo:hi, q0 : q0 + QW], pv[lo:hi, :], rb[lo:hi, :]
                    )

            def _pv_step(p, pv0, pv1, item, nkc):
                kc, qs, w, pt0, pt1 = item
                _mm(nc, pv0[:, qs:QW], vpa[kc][:, 192 * p : 192 * p + 128],
                    pt0[:, 0:w], start=(kc == 0), stop=(kc == nkc - 1))
                _mm(nc, pv1[:, qs:QW], vpa[kc][:, 192 * p + 64 : 192 * p + 192],
                    pt1[:, 0:w], start=(kc == 0), stop=(kc == nkc - 1))

            wo = []

            def load_wo():
                for p4 in range(4):
                    for hf in range(2):
                        t = wp.tile([128, DG], F32R, tag=f"w{p4*2+hf}", name=f"wo{p4*2+hf}")
                        nc.sync.dma_start(
                            out=t[:],
                            in_=WoT[p4 * 128 : (p4 + 1) * 128, hf * 512 : (hf + 1) * 512],
                        )
                        wo.append(t)

            def outproj_block(qi):
                for sub in range(4):
                    sc = qi * 4 + sub
                    for hf in range(2):
                        ps = psS.tile([128, DG], F32, tag="sc")
                        for p4 in range(4):
                            _mm(nc, ps[:], x[p4][:, sc * 128 : (sc + 1) * 128],
                                wo[p4 * 2 + hf][:], start=(p4 == 0), stop=(p4 == 3))
                        st = stg.tile([128, DG], F32, tag="st")
                        nc.vector.tensor_copy(st[:], ps[:])
                        nc.gpsimd.dma_start(
                            out=outp[sc * 128 : (sc + 1) * 128, hf * 512 : (hf + 1) * 512],
                            in_=st[:],
                        )

            # ---------------- phase-major emission ----------------
            with tc.tile_pool(name="ain", bufs=2) as ap2:
                nonlocal_holder["ap_"] = ap2
                wq = load_w(WqT, "q")
                for si in range(4):
                    qin = load_quarter(qT, si)
                    projT_block(qin, wq, qpT, si, bias=bq_sb)
                wk = load_w(WkT, "k")
                for si in range(4):
                    kin = load_quarter(kT, si)
                    projT_block(kin, wk, kpT, si)
                    nat_block(kin, wk, si, "k")
                wv = load_w(WvT, "v")
                for si in range(4):
                    vin = load_quarter(vT, si)
                    nat_block(vin, wv, si, "v")
            load_wo()
            with (
                tc.tile_pool(name="pt", bufs=6) as ptp,
                tc.tile_pool(name="rc", bufs=2) as rcp,
            ):
                for qi in range(NQC):
                    for p in range(4):
                        attn_unit(p, qi, ptp, rcp)
                    outproj_block(qi)
    return nc


_NC_CACHE = {}
_LAST_IN_MAPS = None


def _get_nc():
    if "nc" not in _NC_CACHE:
        nc = bacc.Bacc(
            "TRN2", target_bir_lowering=False, debug=False, num_devices=NCORES
        )
        _emit(nc)
        nc.compile()
        _NC_CACHE["nc"] = nc
    return _NC_CACHE["nc"]


def kernel(q, k, v, mask, Wq, bq, Wk, bk, Wv, bv, Wo, bo):
    q = np.asarray(q, np.float32)
    k = np.asarray(k, np.float32)
    v = np.asarray(v, np.float32)
    Wq = np.asarray(Wq, np.float32)
    Wk = np.asarray(Wk, np.float32)
    Wv = np.asarray(Wv, np.float32)
    Wo = np.asarray(Wo, np.float32)
    bq = np.asarray(bq, np.float32)
    bk = np.asarray(bk, np.float32)
    bv = np.asarray(bv, np.float32)
    bo = np.asarray(bo, np.float32)

    nc = _get_nc()

    tri = np.zeros((128, 128), np.float32)
    iu = np.triu_indices(128, k=1)
    # pT layout is [k, q]: masked iff q < k -> strictly lower triangle of [k, q]
    tri[(iu[1], iu[0])] = NEGMASK

    WqT_f = np.ascontiguousarray(Wq.T)  # [E, D]
    WkT_f = np.ascontiguousarray(Wk.T)
    WvT_f = np.ascontiguousarray(Wv.T)
    WoT_f = np.ascontiguousarray(Wo.T)  # [D(mid), D(out)]

    in_maps = []
    for core in range(NCORES):
        b, g = core // 2, core % 2
        sl = slice(g * DG, (g + 1) * DG)
        in_maps.append(
            {
                "qT": np.ascontiguousarray(q[b].T),
                "kT": np.ascontiguousarray(k[b].T),
                "vT": np.ascontiguousarray(v[b].T),
                "WqT": np.ascontiguousarray(WqT_f[:, sl]),
                "WkT": np.ascontiguousarray(WkT_f[:, sl]),
                "WvT": np.ascontiguousarray(WvT_f[:, sl]),
                "WoT": np.ascontiguousarray(WoT_f[sl, :]),
                "bq": np.ascontiguousarray(bq[sl]),
                "tri": tri,
            }
        )

    global _LAST_IN_MAPS
    _LAST_IN_MAPS = in_maps
    res = run_bass_kernel_spmd(nc, in_maps, list(range(NCORES)))

    out = np.zeros((B, S, D), np.float32)
    kh = np.zeros((B, H, S, HD), np.float32)
    vh = np.zeros((B, H, S, HD), np.float32)
    for core in range(NCORES):
        b, g = core // 2, core % 2
        r = res.results[core]
        out[b] += r["outp"]
        kh[b, g * HG : (g + 1) * HG] = r["kh_o"]
        vh[b, g * HG : (g + 1) * HG] = r["vh_o"]
    # restore biases dropped on-device (linear / softmax-invariant)
    out += bo[None, None, :]
    out += (bv @ Wo.T)[None, None, :]
    kh += bk.reshape(H, 1, HD)
    vh += bv.reshape(H, 1, HD)
    return out, kh, vh


# revision 24
# speedup vs baseline: 1.1656x; 1.0989x over previous
"""Trainium2 Bass kernel for a 16-head causal MHA block (B=4, S=2048, D=1024).

Sharding: 8 cores = 4 batches x 2 head-groups (8 heads each).
Per-core dataflow (all feature-major / "transposed" layouts):
  qpT[d,s] = WqT.T @ qT   (+bq)        kpT[d,s] = WkT.T @ kT
  kp [s,d] = kT.T @ WkT   (kh output)  vp [s,d] = vT.T @ WvT  (vh output + PV)
  scoresT[k,q] = kpT.T(d) @ qpT  (per head, K=64, causal-narrowed)
  pT = exp(scoresT/8 + tri)      (no max-subtract: scores ~ N(0,1))
  pv[d,q] & rowsum via ones-augmented stationary [v|1] (M=128)
  x[d,q] = pv * (1/rowsum)       out_part[s,:] = x.T(d) @ WoT
Biases bk, bv, bo are restored on the host (softmax-invariant / linear).
Emission is interleaved si-quarter -> attention qi -> outproj qi so the
Tile scheduler can overlap projections (PE) with attention (ACT-paced).
"""
import numpy as np

import concourse.bass as bass
import concourse.mybir as mybir
import concourse.tile as tile
from concourse import bacc
from concourse.bass_utils import run_bass_kernel_spmd

F32 = mybir.dt.float32
F32R = mybir.dt.float32r
AF = mybir.ActivationFunctionType

B, S, D, H = 4, 2048, 1024, 16
HD = 64
NCORES = 8
HG = 8      # heads per core
DG = 512    # projected dims per core (head-group)
E = 1024    # input feature dim
SCALE = 0.125       # 1/sqrt(HD)
NEGMASK = -8.0e5    # pre-scale additive mask: exp((s + NEGMASK)/8) == 0.0 in f32

QW = 512    # q-chunk width for attention
NQC = S // QW   # 4
NKC = S // 128  # 16


def _mm(nc, out, lhsT, rhs, start, stop):
    nc.tensor.matmul(out, lhsT=lhsT, rhs=rhs, start=start, stop=stop)


def _emit(nc):
    qT = nc.dram_tensor("qT", [E, S], F32R, kind="ExternalInput").ap()
    kT = nc.dram_tensor("kT", [E, S], F32R, kind="ExternalInput").ap()
    vT = nc.dram_tensor("vT", [E, S], F32R, kind="ExternalInput").ap()
    WqT = nc.dram_tensor("WqT", [E, DG], F32R, kind="ExternalInput").ap()
    WkT = nc.dram_tensor("WkT", [E, DG], F32R, kind="ExternalInput").ap()
    WvT = nc.dram_tensor("WvT", [E, DG], F32R, kind="ExternalInput").ap()
    WoT = nc.dram_tensor("WoT", [DG, D], F32R, kind="ExternalInput").ap()
    bqv = nc.dram_tensor("bq", [DG], F32, kind="ExternalInput").ap()
    tri = nc.dram_tensor("tri", [128, 128], F32, kind="ExternalInput").ap()
    outp = nc.dram_tensor("outp", [S, D], F32, kind="ExternalOutput").ap()
    kh_o = nc.dram_tensor("kh_o", [HG, S, HD], F32, kind="ExternalOutput").ap()
    vh_o = nc.dram_tensor("vh_o", [HG, S, HD], F32, kind="ExternalOutput").ap()

    with tile.TileContext(nc) as tc:
        with (
            tc.tile_pool(name="persist", bufs=1) as pp,
            tc.tile_pool(name="win", bufs=1) as wp,
            tc.tile_pool(name="stage", bufs=2) as stg,
            tc.tile_pool(name="psS", bufs=5, space="PSUM") as psS,
            tc.tile_pool(name="psV", bufs=3, space="PSUM") as psV,
        ):
            # ---- persistent tiles ----
            qpT = [pp.tile([128, S], F32R, tag=f"qpT{p}", name=f"qpT{p}") for p in range(4)]
            kpT = [pp.tile([128, S], F32R, tag=f"kpT{p}", name=f"kpT{p}") for p in range(4)]
            # vpa: per s-chunk of 128, interleaved per pair: [v_even|ones|v_odd]
            vpa = [pp.tile([128, 768], F32R, tag=f"vpa{i}", name=f"vpa{i}") for i in range(NKC)]
            x = [pp.tile([128, S], F32R, tag=f"x{p}", name=f"x{p}") for p in range(4)]
            tri_sb = pp.tile([128, 128], F32, tag="tri")
            bq_sb = pp.tile([128, 4], F32, tag="bq")
            ones_sb = pp.tile([128, 256], F32, tag="ones")

            nc.sync.dma_start(out=tri_sb[:], in_=tri)
            nc.sync.dma_start(out=bq_sb[:], in_=bqv.rearrange("(c p) -> p c", p=128))
            nc.gpsimd.memset(ones_sb[:], 1.0)
            for i in range(NKC):
                nc.vector.tensor_copy(
                    vpa[i][:].rearrange("p (j b) -> p j b", j=4)[:, :, 64:128],
                    ones_sb[:].rearrange("p (j b) -> p j b", j=4),
                )

            def load_w(src, pref):
                tiles = []
                for e in range(8):
                    t = wp.tile([128, DG], F32R, tag=f"w{e}", name=f"w{pref}{e}")
                    nc.sync.dma_start(out=t[:], in_=src[e * 128 : (e + 1) * 128, :])
                    tiles.append(t)
                return tiles


            nonlocal_holder = {}

            def load_quarter(src, si):
                tiles = []
                for e in range(8):
                    t = nonlocal_holder["ap_"].tile([128, QW], F32R, tag=f"a{e}", name=f"a{e}")
                    nc.sync.dma_start(
                        out=t[:],
                        in_=src[e * 128 : (e + 1) * 128, si * QW : (si + 1) * QW],
                    )
                    tiles.append(t)
                return tiles

            # ---------------- phase emitters ----------------
            def projT_block(inp, w, out_tiles, si, bias=None):
                for dc in range(4):
                    ps = psS.tile([128, QW], F32, tag="sc")
                    for e in range(8):
                        _mm(nc, ps[:], w[e][:, dc * 128 : (dc + 1) * 128],
                            inp[e][:], start=(e == 0), stop=(e == 7))
                    dst = out_tiles[dc][:, si * QW : (si + 1) * QW]
                    if bias is not None:
                        nc.vector.tensor_scalar_add(dst, ps[:], bias[:, dc : dc + 1])
                    else:
                        nc.vector.tensor_copy(dst, ps[:])

            def nat_block(inp, w, si, kind):
                for sub in range(4):
                    sc = si * 4 + sub
                    ps = psS.tile([128, DG], F32, tag="sc")
                    for e in range(8):
                        _mm(nc, ps[:], inp[e][:, sub * 128 : (sub + 1) * 128],
                            w[e][:], start=(e == 0), stop=(e == 7))
                    if kind == "k":
                        st = stg.tile([128, DG], F32, tag="st")
                        nc.vector.tensor_copy(st[:], ps[:])
                        nc.gpsimd.dma_start(
                            out=kh_o.rearrange("h s c -> s h c")[sc * 128 : (sc + 1) * 128],
                            in_=st[:].rearrange("p (h c) -> p h c", c=HD),
                        )
                    else:
                        nc.vector.tensor_copy(
                            vpa[sc][:].rearrange("p (j b) -> p j b", j=4)[:, :, 0:64],
                            ps[:].rearrange("p (j b) -> p j b", j=4)[:, :, 0:64],
                        )
                        nc.vector.tensor_copy(
                            vpa[sc][:].rearrange("p (j b) -> p j b", j=4)[:, :, 128:192],
                            ps[:].rearrange("p (j b) -> p j b", j=4)[:, :, 64:128],
                        )
                        stv = stg.tile([128, DG], F32, tag="st")
                        nc.vector.tensor_copy(stv[:], ps[:])
                        nc.gpsimd.dma_start(
                            out=vh_o.rearrange("h s c -> s h c")[sc * 128 : (sc + 1) * 128],
                            in_=stv[:].rearrange("p (h c) -> p h c", c=HD),
                        )

            def attn_unit(p, qi, ptp, rcp):
                q0 = qi * QW
                nkc = (q0 + QW) // 128
                pv0 = psV.tile([128, QW], F32, tag="pv")
                pv1 = psV.tile([128, QW], F32, tag="pv")
                pend = []  # software pipeline: PV trails scores by one kc
                for kc in range(nkc):
                    k0 = kc * 128
                    qs = max(0, k0 - q0)
                    w = QW - qs
                    s0 = psS.tile([128, QW], F32, tag="sc")
                    s1 = psS.tile([128, QW], F32, tag="sc")
                    _mm(nc, s0[:, qs:QW], kpT[p][0:64, k0 : k0 + 128],
                        qpT[p][0:64, q0 + qs : q0 + QW], start=True, stop=True)
                    _mm(nc, s1[:, qs:QW], kpT[p][64:128, k0 : k0 + 128],
                        qpT[p][64:128, q0 + qs : q0 + QW], start=True, stop=True)
                    if k0 >= q0:
                        nc.vector.tensor_add(s0[:, qs : qs + 128], s0[:, qs : qs + 128], tri_sb[:])
                        nc.vector.tensor_add(s1[:, qs : qs + 128], s1[:, qs : qs + 128], tri_sb[:])
                    pt0 = ptp.tile([128, QW], F32R, tag="pt", name="pt0")
                    pt1 = ptp.tile([128, QW], F32R, tag="pt", name="pt1")
                    nc.scalar.activation(pt0[:, 0:w], s0[:, qs:QW], AF.Exp, scale=SCALE)
                    nc.scalar.activation(pt1[:, 0:w], s1[:, qs:QW], AF.Exp, scale=SCALE)
                    pend.append((kc, qs, w, pt0, pt1))
                    if len(pend) > 2:
                        _pv_step(p, pv0, pv1, pend.pop(0), nkc)
                while pend:
                    _pv_step(p, pv0, pv1, pend.pop(0), nkc)
                # normalization / eviction
                for j, pv in ((0, pv0), (1, pv1)):
                    lo, hi = (0, 64) if j == 0 else (64, 128)
                    slo, shi = (64, 128) if j == 0 else (0, 64)
                    r = rcp.tile([128, QW], F32, tag="rc")
                    nc.vector.reciprocal_approx_fast(r[:], pv[:])
                    rb = rcp.tile([128, QW], F32, tag="rcb")
                    nc.gpsimd.dma_start(out=rb[lo:hi, :], in_=r[slo:shi, :])
                    nc.vector.tensor_mul(
                        x[p][lo:hi, q0 : q0 + QW], pv[lo:hi, :], rb[lo:hi, :]
                    )

            def _pv_step(p, pv0, pv1, item, nkc):
                kc, qs, w, pt0, pt1 = item
                _mm(nc, pv0[:, qs:QW], vpa[kc][:, 192 * p : 192 * p + 128],
                    pt0[:, 0:w], start=(kc == 0), stop=(kc == nkc - 1))
                _mm(nc, pv1[:, qs:QW], vpa[kc][:, 192 * p + 64 : 192 * p + 192],
                    pt1[:, 0:w], start=(kc == 0), stop=(kc == nkc - 1))

            wo = []

            def load_wo():
                for p4 in range(4):
                    for hf in range(2):
                        t = wp.tile([128, DG], F32R, tag=f"w{p4*2+hf}", name=f"wo{p4*2+hf}")
                        nc.sync.dma_start(
                            out=t[:],
                            in_=WoT[p4 * 128 : (p4 + 1) * 128, hf * 512 : (hf + 1) * 512],
                        )
                        wo.append(t)

            def outproj_block(qi):
                for sub in range(4):
                    sc = qi * 4 + sub
                    for hf in range(2):
                        ps = psS.tile([128, DG], F32, tag="sc")
                        for p4 in range(4):
                            _mm(nc, ps[:], x[p4][:, sc * 128 : (sc + 1) * 128],
                                wo[p4 * 2 + hf][:], start=(p4 == 0), stop=(p4 == 3))
                        st = stg.tile([128, DG], F32, tag="st")
                        nc.vector.tensor_copy(st[:], ps[:])
                        nc.gpsimd.dma_start(
                            out=outp[sc * 128 : (sc + 1) * 128, hf * 512 : (hf + 1) * 512],
                            in_=st[:],
                        )

            # ---------------- phase-major emission ----------------
            with tc.tile_pool(name="ain", bufs=2) as ap2:
                nonlocal_holder["ap_"] = ap2
                wq = load_w(WqT, "q")
                for si in range(4):
                    qin = load_quarter(qT, si)
                    projT_block(qin, wq, qpT, si, bias=bq_sb)
                wk = load_w(WkT, "k")
                for si in range(4):
                    kin = load_quarter(kT, si)
                    projT_block(kin, wk, kpT, si)
                    nat_block(kin, wk, si, "k")
                wv = load_w(WvT, "v")
                for si in range(4):
                    vin = load_quarter(vT, si)
                    nat_block(vin, wv, si, "v")
            load_wo()
            with (
                tc.tile_pool(name="pt", bufs=6) as ptp,
                tc.tile_pool(name="rc", bufs=2) as rcp,
            ):
                for qi in range(NQC):
                    for p in range(4):
                        attn_unit(p, qi, ptp, rcp)
                    outproj_block(qi)
    return nc


_NC_CACHE = {}
_LAST_IN_MAPS = None


def _get_nc():
    if "nc" not in _NC_CACHE:
        nc = bacc.Bacc(
            "TRN2", target_bir_lowering=False, debug=False, num_devices=NCORES
        )
        _emit(nc)
        nc.compile()
        _NC_CACHE["nc"] = nc
    return _NC_CACHE["nc"]


def kernel(q, k, v, mask, Wq, bq, Wk, bk, Wv, bv, Wo, bo):
    q = np.asarray(q, np.float32)
    k = np.asarray(k, np.float32)
    v = np.asarray(v, np.float32)
    Wq = np.asarray(Wq, np.float32)
    Wk = np.asarray(Wk, np.float32)
    Wv = np.asarray(Wv, np.float32)
    Wo = np.asarray(Wo, np.float32)
    bq = np.asarray(bq, np.float32)
    bk = np.asarray(bk, np.float32)
    bv = np.asarray(bv, np.float32)
    bo = np.asarray(bo, np.float32)

    nc = _get_nc()

    tri = np.zeros((128, 128), np.float32)
    iu = np.triu_indices(128, k=1)
    # pT layout is [k, q]: masked iff q < k -> strictly lower triangle of [k, q]
    tri[(iu[1], iu[0])] = NEGMASK

    WqT_f = np.ascontiguousarray(Wq.T)  # [E, D]
    WkT_f = np.ascontiguousarray(Wk.T)
    WvT_f = np.ascontiguousarray(Wv.T)
    WoT_f = np.ascontiguousarray(Wo.T)  # [D(mid), D(out)]

    in_maps = []
    for core in range(NCORES):
        b, g = core // 2, core % 2
        sl = slice(g * DG, (g + 1) * DG)
        in_maps.append(
            {
                "qT": np.ascontiguousarray(q[b].T),
                "kT": np.ascontiguousarray(k[b].T),
                "vT": np.ascontiguousarray(v[b].T),
                "WqT": np.ascontiguousarray(WqT_f[:, sl]),
                "WkT": np.ascontiguousarray(WkT_f[:, sl]),
                "WvT": np.ascontiguousarray(WvT_f[:, sl]),
                "WoT": np.ascontiguousarray(WoT_f[sl, :]),
                "bq": np.ascontiguousarray(bq[sl]),
                "tri": tri,
            }
        )

    global _LAST_IN_MAPS
    _LAST_IN_MAPS = in_maps
    res = run_bass_kernel_spmd(nc, in_maps, list(range(NCORES)))

    out = np.zeros((B, S, D), np.float32)
    kh = np.zeros((B, H, S, HD), np.float32)
    vh = np.zeros((B, H, S, HD), np.float32)
    for core in range(NCORES):
        b, g = core // 2, core % 2
        r = res.results[core]
        out[b] += r["outp"]
        kh[b, g * HG : (g + 1) * HG] = r["kh_o"]
        vh[b, g * HG : (g + 1) * HG] = r["vh_o"]
    # restore biases dropped on-device (linear / softmax-invariant)
    out += bo[None, None, :]
    out += (bv @ Wo.T)[None, None, :]
    kh += bk.reshape(H, 1, HD)
    vh += bv.reshape(H, 1, HD)
    return out, kh, vh
